# revision 2
# baseline (speedup 1.0000x reference)
"""Multi-head self-attention (B=2, T=2048, D=1024, 16 heads) on 8 TRN2 cores.

Sharding: core c = (b, g) with b = c // 4 (batch), g = c % 4 (head group of 4).
Each core computes q/k/v projections for its 4 heads, causal softmax
attention, and a partial output projection (its 256 columns of the
concat-head dim against Wo). Host sums the 4 partials per batch and adds bo.

On-device layouts (per core):
  HT      [1024, 2048]  H[b].T            (K=D on partitions for projections)
  qT, kT  [256, 2048]   q/k transposed, 2 SBUF tiles of [128, 2048]
                        (tile m holds heads 2m, 2m+1 on partitions 0-63/64-127)
  v       [2048, 260]   natural, 16 tiles [128, 260]; per head 65 cols:
                        64 value cols + a ones col (softmax denominators)
  scoresT [128, 512]    PSUM per (head, key-chunk kc, query-block J):
                        kT_h[:, kc].T @ qT_h[:, J]  -- row-packed pairs of
                        heads run concurrently on the PE (base partition 0/64)
  exp     [128, 512]    ACT: exp(0.125 * scoresT), fp32r; causal-masked on the
                        4 diagonal chunks via DVE multiply with static masks
  attT    [65, 512]     PSUM: [v_h | ones].T @ exp, accumulated over kc;
                        row 64 = softmax denominators
  attT_n  [256, 2048]   normalized attT (DVE mult by broadcast reciprocal)
  O       [2048, 1024]  attT_n.T @ WoS (partial, host adds across head groups)

All matmuls run in float32r (TF32-like, full PE rate at free dim >= 256).
"""

import numpy as np

import concourse.bass as bass
import concourse.tile as tile
from concourse import bacc, mybir
from concourse import bass_utils
from contextlib import ExitStack

F32 = mybir.dt.float32
F32R = mybir.dt.float32r
AF = mybir.ActivationFunctionType
OP = mybir.AluOpType

B, T, D = 2, 2048, 1024
NH, DH = 16, 64
HPC = 4            # heads per core
GD = HPC * DH      # 256, group dim
GV = HPC * (DH + 1)  # 260, v tile width (64 value cols + ones col per head)
NKD = D // 128     # 8 K-chunks for projections
NT = T // 128      # 16 token chunks
NJ = T // 512      # 4 query blocks

_NC_CACHE = {}


def build():
    if "nc" in _NC_CACHE:
        return _NC_CACHE["nc"]
    nc = bacc.Bacc("TRN2", target_bir_lowering=False, debug=False, num_devices=8)

    HT = nc.dram_tensor("HT", [D, T], F32, kind="ExternalInput").ap()
    WqT = nc.dram_tensor("WqT", [D, GD], F32, kind="ExternalInput").ap()
    WkT = nc.dram_tensor("WkT", [D, GD], F32, kind="ExternalInput").ap()
    WvS = nc.dram_tensor("WvS", [D, GV], F32, kind="ExternalInput").ap()
    WoS = nc.dram_tensor("WoS", [GD, D], F32, kind="ExternalInput").ap()
    bq = nc.dram_tensor("bq", [1, GD], F32, kind="ExternalInput").ap()
    bk = nc.dram_tensor("bk", [1, GD], F32, kind="ExternalInput").ap()
    bvS = nc.dram_tensor("bvS", [1, GV], F32, kind="ExternalInput").ap()
    kpm = nc.dram_tensor("kpm", [128, NT], F32, kind="ExternalInput").ap()
    O = nc.dram_tensor("O", [T, D], F32, kind="ExternalOutput").ap()

    with tile.TileContext(nc) as tc, ExitStack() as octx:
        cpool = octx.enter_context(tc.tile_pool(name="const", bufs=1))
        keep = octx.enter_context(tc.tile_pool(name="keep", bufs=1))

        # ---- constants ----
        ones_f = cpool.tile([1, 512], F32, name="ones_f", tag="ones_f")
        nc.vector.memset(ones_f[:], 1.0)
        ones_r = cpool.tile([1, 512], F32R, name="ones_r", tag="ones_r")
        nc.vector.tensor_copy(ones_r[:], ones_f[:])

        # bias rows -> fp32r
        bstage = cpool.tile([1, GV], F32, name="bstage", tag="bstage", bufs=3)
        bq_r = cpool.tile([1, GD], F32R, name="bq_r", tag="bq_r")
        bk_r = cpool.tile([1, GD], F32R, name="bk_r", tag="bk_r")
        bv_r = cpool.tile([1, GV], F32R, name="bv_r", tag="bv_r")
        for src, dst, w in ((bq, bq_r, GD), (bk, bk_r, GD), (bvS, bv_r, GV)):
            st = cpool.tile([1, GV], F32, name="bstage", tag="bstage", bufs=3)
            nc.sync.dma_start(st[:, 0:w], src[:])
            nc.vector.tensor_copy(dst[:], st[:, 0:w])

        kpm_sb = cpool.tile([128, NT], F32, name="kpm_sb", tag="kpm_sb")
        nc.sync.dma_start(kpm_sb[:], kpm[:])

        # causal masks for the 4 diagonal chunks: keep where f >= p + 128*i
        masks = []
        for i in range(4):
            m = cpool.tile([128, 512], F32, name=f"mask{i}", tag=f"mask{i}")
            nc.gpsimd.memset(m[:], 1.0)
            nc.gpsimd.affine_select(
                out=m[:], in_=m[:], compare_op=OP.is_ge, fill=0.0,
                base=-128 * i, pattern=[[1, 512]], channel_multiplier=-1,
            )
            masks.append(m)

        # ---- long-lived activations ----
        qT = [keep.tile([128, T], F32R, name=f"qT{m}", tag=f"qT{m}") for m in range(2)]
        kT = [keep.tile([128, T], F32R, name=f"kT{m}", tag=f"kT{m}") for m in range(2)]
        vt = [keep.tile([128, GV], F32R, name=f"vt{t}", tag=f"vt{t}") for t in range(NT)]
        attT = [keep.tile([128, T], F32R, name=f"attT{m}", tag=f"attT{m}") for m in range(2)]
        wo_r = [keep.tile([128, D], F32R, name=f"wo{i}", tag=f"wo{i}") for i in range(2)]

        # ================= phase A: projections =================
        with ExitStack() as actx:
            apool = actx.enter_context(tc.tile_pool(name="phA", bufs=1))
            apsum = actx.enter_context(tc.tile_pool(name="phA_ps", bufs=1, space="PSUM"))

            ht_r = [apool.tile([128, T], F32R, name=f"ht{k}", tag=f"ht{k}") for k in range(NKD)]
            for k in range(NKD):
                st = apool.tile([128, T], F32, name="hstage", tag="hstage", bufs=2)
                nc.sync.dma_start(st[:], HT[k * 128:(k + 1) * 128, :])
                nc.vector.tensor_copy(ht_r[k][:], st[:])

            wq_r = apool.tile([128, NKD * GD], F32R, name="wq_r", tag="wq_r")
            wk_r = apool.tile([128, NKD * GD], F32R, name="wk_r", tag="wk_r")
            wv_r = apool.tile([128, NKD * GV], F32R, name="wv_r", tag="wv_r")
            for src, dst, w in ((WqT, wq_r, GD), (WkT, wk_r, GD), (WvS, wv_r, GV)):
                for k in range(NKD):
                    ws = apool.tile([128, GV], F32, name="wstage", tag="wstage", bufs=4)
                    nc.sync.dma_start(ws[:, 0:w], src[k * 128:(k + 1) * 128, :])
                    nc.vector.tensor_copy(dst[:, k * w:(k + 1) * w], ws[:, 0:w])
            for i in range(2):
                ws = apool.tile([128, D], F32, name="wostage", tag="wostage", bufs=2)
                nc.sync.dma_start(ws[:], WoS[i * 128:(i + 1) * 128, :])
                nc.vector.tensor_copy(wo_r[i][:], ws[:])

            # qT / kT: out[dq(128), t(512)] = sum_k WT_k_m.T @ HT_k + bias
            for w_r, dest, brow in ((wq_r, qT, bq_r), (wk_r, kT, bk_r)):
                for m in range(2):
                    ps = [
                        apsum.tile([128, 512], F32, name=f"pp{n}", tag="proj", bufs=6)
                        for n in range(4)
                    ]
                    for k in range(NKD):
                        for n in range(4):
                            nc.tensor.matmul(
                                ps[n][:],
                                w_r[:, k * GD + m * 128: k * GD + m * 128 + 128],
                                ht_r[k][:, n * 512:(n + 1) * 512],
                                start=(k == 0), stop=False,
                            )
                    for n in range(4):
                        # bias: ones.T @ bias_row chunk (K=1)
                        nc.tensor.matmul(
                            ps[n][:],
                            brow[:, m * 128:(m + 1) * 128],
                            ones_r[:],
                            start=False, stop=True,
                        )
                        nc.scalar.copy(dest[m][:, n * 512:(n + 1) * 512], ps[n][:])

            # v natural: out[t(128), 260] = sum_k HT_k_t.T @ WvS_k + ones.T @ bvS
            for t in range(NT):
                vp = apsum.tile([128, GV], F32, name="vp", tag="vps", bufs=2)
                for k in range(NKD):
                    nc.tensor.matmul(
                        vp[:],
                        ht_r[k][:, t * 128:(t + 1) * 128],
                        wv_r[:, k * GV:(k + 1) * GV],
                        start=(k == 0), stop=False,
                    )
                nc.tensor.matmul(
                    vp[:], ones_r[:, 0:128], bv_r[:], start=False, stop=True
                )
                nc.scalar.copy(vt[t][:], vp[:])
                nc.vector.tensor_scalar_mul(vt[t][:], vt[t][:], kpm_sb[:, t:t + 1])

        # ================= phase B: attention =================
        with ExitStack() as bctx:
            bpool = bctx.enter_context(tc.tile_pool(name="phB", bufs=1))
            bpsum = bctx.enter_context(tc.tile_pool(name="phB_ps", bufs=1, space="PSUM"))

            for J in range(NJ):
                for hp in range(2):
                    n_kc = 4 * J + 4
                    at = [
                        bpsum.tile([65, 512], F32, name=f"at{hh}", tag=f"av{hh}", bufs=2)
                        for hh in range(2)
                    ]
                    for kc in range(n_kc):
                        for hh in range(2):
                            h = 2 * hp + hh
                            sc = bpsum.tile([128, 512], F32, name="sc", tag="sc", bufs=4)
                            nc.tensor.matmul(
                                sc[:],
                                kT[hp][hh * 64:(hh + 1) * 64, kc * 128:(kc + 1) * 128],
                                qT[hp][hh * 64:(hh + 1) * 64, J * 512:(J + 1) * 512],
                                start=True, stop=True,
                            )
                            ex = bpool.tile([128, 512], F32R, name="ex", tag="ex", bufs=4)
                            nc.scalar.activation(ex[:], sc[:], AF.Exp, scale=0.125)
                            if kc >= 4 * J:
                                nc.vector.tensor_tensor(
                                    ex[:], ex[:], masks[kc - 4 * J][:], op=OP.mult
                                )
                            nc.tensor.matmul(
                                at[hh][:],
                                vt[kc][:, h * 65:(h + 1) * 65],
                                ex[:],
                                start=(kc == 0), stop=(kc == n_kc - 1),
                            )
                    for hh in range(2):
                        rc = bpool.tile([1, 512], F32, name="rc", tag="rc", bufs=4)
                        nc.vector.reciprocal(rc[:], at[hh][64:65, :])
                        rb = bpool.tile([64, 512], F32, name="rb", tag="rb", bufs=4)
                        nc.gpsimd.partition_broadcast(rb[:], rc[:])
                        nc.vector.tensor_tensor(
                            attT[hp][hh * 64:(hh + 1) * 64, J * 512:(J + 1) * 512],
                            at[hh][0:64, :],
                            rb[:],
                            op=OP.mult,
                        )

        # ================= phase C: output projection =================
        with ExitStack() as cctx:
            opool = cctx.enter_context(tc.tile_pool(name="phC", bufs=1))
            opsum = cctx.enter_context(tc.tile_pool(name="phC_ps", bufs=1, space="PSUM"))
            for t in range(NT):
                ot = opool.tile([128, D], F32, name="ot", tag="ot", bufs=3)
                for n in range(2):
                    op = opsum.tile([128, 512], F32, name="op", tag="op", bufs=4)
                    for hp in range(2):
                        nc.tensor.matmul(
                            op[:],
                            attT[hp][:, t * 128:(t + 1) * 128],
                            wo_r[hp][:, n * 512:(n + 1) * 512],
                            start=(hp == 0), stop=(hp == 1),
                        )
                    nc.scalar.copy(ot[:, n * 512:(n + 1) * 512], op[:])
                nc.sync.dma_start(O[t * 128:(t + 1) * 128, :], ot[:])

    nc.compile()
    _NC_CACHE["nc"] = nc
    return nc


def _prep_core_inputs(H, key_padding_mask, Wq, bq, Wk, bk, Wv, bv, Wo, bo):
    keep = 1.0 - np.asarray(key_padding_mask, dtype=np.float32)  # [B, T]
    in_maps = []
    for c in range(8):
        b, g = divmod(c, 4)
        sl = slice(g * GD, (g + 1) * GD)
        WvT = Wv[sl].T  # [D, GD]
        WvS = np.zeros((D, GV), dtype=np.float32)
        bvS = np.zeros((1, GV), dtype=np.float32)
        for h in range(HPC):
            WvS[:, h * 65:h * 65 + 64] = WvT[:, h * 64:(h + 1) * 64]
            bvS[0, h * 65:h * 65 + 64] = bv[sl][h * 64:(h + 1) * 64]
            bvS[0, h * 65 + 64] = 1.0
        in_maps.append({
            "HT": np.ascontiguousarray(H[b].T),
            "WqT": np.ascontiguousarray(Wq[sl].T),
            "WkT": np.ascontiguousarray(Wk[sl].T),
            "WvS": WvS,
            "WoS": np.ascontiguousarray(Wo[:, sl].T),
            "bq": np.ascontiguousarray(bq[sl][None, :]),
            "bk": np.ascontiguousarray(bk[sl][None, :]),
            "bvS": bvS,
            "kpm": np.ascontiguousarray(keep[b].reshape(NT, 128).T),
        })
    return in_maps


def kernel(H, key_padding_mask, Wq, bq, Wk, bk, Wv, bv, Wo, bo, _run_kwargs=None):
    H = np.asarray(H, dtype=np.float32)
    Wq = np.asarray(Wq, dtype=np.float32)
    Wk = np.asarray(Wk, dtype=np.float32)
    Wv = np.asarray(Wv, dtype=np.float32)
    Wo = np.asarray(Wo, dtype=np.float32)
    bq = np.asarray(bq, dtype=np.float32)
    bk = np.asarray(bk, dtype=np.float32)
    bv = np.asarray(bv, dtype=np.float32)
    bo = np.asarray(bo, dtype=np.float32)

    nc = build()
    in_maps = _prep_core_inputs(H, key_padding_mask, Wq, bq, Wk, bk, Wv, bv, Wo, bo)
    res = bass_utils.run_bass_kernel_spmd(
        nc, in_maps, core_ids=list(range(8)), **(_run_kwargs or {})
    )
    out = np.zeros((B, T, D), dtype=np.float32)
    for c in range(8):
        out[c // 4] += res.results[c]["O"]
    out += bo
    if _run_kwargs:
        kernel.last_result = res
    return out


# revision 6
# speedup vs baseline: 1.0749x; 1.0749x over previous
"""Multi-head self-attention (B=2, T=2048, D=1024, 16 heads) on 8 TRN2 cores.

Sharding: core c = (b, g) with b = c // 4 (batch), g = c % 4 (head group of 4).
Each core computes q/k/v projections for its 4 heads, causal softmax
attention, and a partial output projection (its 256 columns of the
concat-head dim against Wo). Host sums the 4 partials per batch and adds bo.

All matmuls run in float32r (TF32-like, full PE rate at free dim >= 256).
DRAM inputs are declared float32r so they DMA straight into matmul operands.

Per-core pipeline:
  phase A: qT/kT [256,2048] (transposed projections, head pairs stacked on
           partitions) and v [2048,260] natural (per head 64 value cols + a
           ones col that makes the AV matmul emit softmax denominators).
  phase B: per (head-pair hp, query block J of 512): scoresT chunks
           [tk=128, tq<=512] = kT.T @ qT row-packed via tile_position
           (0,0)/(64,0); ACT exp(0.125 x) PSUM->SBUF (off-diagonal chunks
           paired into [128,1024] PSUM tiles, one ACT instr); diagonal chunks
           column-restricted to the causal region and masked with a single
           [128,128] lower-tri multiply; AV accumulates [v|1].T @ exp into
           attT [65, 512]; row 64 holds denominators; normalize via
           reciprocal_approx_fast + gpsimd partition broadcast + DVE mult.
  phase C: O [2048,1024] = attT.T @ WoS, DVE PSUM->SBUF copies, DMA out.
"""

import numpy as np

import concourse.bass as bass
import concourse.tile as tile
from concourse import bacc, mybir
from concourse import bass_utils
from contextlib import ExitStack

F32 = mybir.dt.float32
F32R = mybir.dt.float32r
AF = mybir.ActivationFunctionType
OP = mybir.AluOpType

B, T, D = 2, 2048, 1024
NH, DH = 16, 64
HPC = 4            # heads per core
GD = HPC * DH      # 256, group dim
GV = HPC * (DH + 1)  # 260, v tile width
NKD = D // 128     # 8 K-chunks for projections
NT = T // 128      # 16 token chunks
NJ = T // 512      # 4 query blocks

_NC_CACHE = {}


def build():
    if "nc" in _NC_CACHE:
        return _NC_CACHE["nc"]
    nc = bacc.Bacc("TRN2", target_bir_lowering=False, debug=False, num_devices=8)

    HT = nc.dram_tensor("HT", [D, T], F32R, kind="ExternalInput").ap()
    WqT = nc.dram_tensor("WqT", [D, GD], F32R, kind="ExternalInput").ap()
    WkT = nc.dram_tensor("WkT", [D, GD], F32R, kind="ExternalInput").ap()
    WvS = nc.dram_tensor("WvS", [D, GV], F32R, kind="ExternalInput").ap()
    WoS = nc.dram_tensor("WoS", [GD, D], F32R, kind="ExternalInput").ap()
    bq = nc.dram_tensor("bq", [1, GD], F32R, kind="ExternalInput").ap()
    bk = nc.dram_tensor("bk", [1, GD], F32R, kind="ExternalInput").ap()
    bvS = nc.dram_tensor("bvS", [1, GV], F32R, kind="ExternalInput").ap()
    kpm = nc.dram_tensor("kpm", [128, NT], F32, kind="ExternalInput").ap()
    O = nc.dram_tensor("O", [T, D], F32, kind="ExternalOutput").ap()

    ENGS = [nc.sync, nc.scalar, nc.gpsimd]

    with tile.TileContext(nc) as tc, ExitStack() as octx:
        cpool = octx.enter_context(tc.tile_pool(name="const", bufs=1))
        keep = octx.enter_context(tc.tile_pool(name="keep", bufs=1))

        # ---- constants ----
        ones_f = cpool.tile([1, 512], F32, name="ones_f", tag="ones_f")
        nc.vector.memset(ones_f[:], 1.0)
        ones_r = cpool.tile([1, 512], F32R, name="ones_r", tag="ones_r")
        nc.vector.tensor_copy(ones_r[:], ones_f[:])

        bq_r = cpool.tile([1, GD], F32R, name="bq_r", tag="bq_r")
        bk_r = cpool.tile([1, GD], F32R, name="bk_r", tag="bk_r")
        bv_r = cpool.tile([1, GV], F32R, name="bv_r", tag="bv_r")
        nc.sync.dma_start(bq_r[:], bq[:])
        nc.sync.dma_start(bk_r[:], bk[:])
        nc.sync.dma_start(bv_r[:], bvS[:])

        kpm_sb = cpool.tile([128, NT], F32, name="kpm_sb", tag="kpm_sb")
        nc.sync.dma_start(kpm_sb[:], kpm[:])

        # lower-tri mask [128,128]: keep where f >= p
        tri = cpool.tile([128, 128], F32, name="tri", tag="tri")
        nc.gpsimd.memset(tri[:], 1.0)
        nc.gpsimd.affine_select(
            out=tri[:], in_=tri[:], compare_op=OP.is_ge, fill=0.0,
            base=0, pattern=[[1, 128]], channel_multiplier=-1,
        )

        # ---- long-lived activations ----
        qT = [keep.tile([128, T], F32R, name=f"qT{m}", tag=f"qT{m}") for m in range(2)]
        kT = [keep.tile([128, T], F32R, name=f"kT{m}", tag=f"kT{m}") for m in range(2)]
        vt = [keep.tile([128, GV], F32R, name=f"vt{t}", tag=f"vt{t}") for t in range(NT)]
        attT = [keep.tile([128, T], F32R, name=f"attT{m}", tag=f"attT{m}") for m in range(2)]
        wo_r = [keep.tile([128, D], F32R, name=f"wo{i}", tag=f"wo{i}") for i in range(2)]
        for i in range(2):
            nc.scalar.dma_start(wo_r[i][:], WoS[i * 128:(i + 1) * 128, :])

        # ================= phase A: projections =================
        with ExitStack() as actx:
            apool = actx.enter_context(tc.tile_pool(name="phA", bufs=1))
            apsum = actx.enter_context(tc.tile_pool(name="phA_ps", bufs=1, space="PSUM"))

            ht_r = [apool.tile([128, T], F32R, name=f"ht{k}", tag=f"ht{k}") for k in range(NKD)]
            wq_r = apool.tile([128, NKD * GD], F32R, name="wq_r", tag="wq_r")
            wk_r = apool.tile([128, NKD * GD], F32R, name="wk_r", tag="wk_r")
            wv_r = apool.tile([128, NKD * GV], F32R, name="wv_r", tag="wv_r")
            # interleave weight + HT chunk loads across engine DMA queues
            for k in range(NKD):
                e = ENGS[k % len(ENGS)]
                e.dma_start(wq_r[:, k * GD:(k + 1) * GD], WqT[k * 128:(k + 1) * 128, :])
                e.dma_start(wk_r[:, k * GD:(k + 1) * GD], WkT[k * 128:(k + 1) * 128, :])
                e.dma_start(wv_r[:, k * GV:(k + 1) * GV], WvS[k * 128:(k + 1) * 128, :])
                ENGS[(k + 2) % len(ENGS)].dma_start(ht_r[k][:], HT[k * 128:(k + 1) * 128, :])

            # qT / kT: out[dq(128), t(512)] = sum_k WT_k_m.T @ HT_k + bias
            for w_r, dest, brow in ((wq_r, qT, bq_r), (wk_r, kT, bk_r)):
                for m in range(2):
                    ps = [
                        apsum.tile([128, 512], F32, name=f"pp{n}", tag="proj", bufs=6)
                        for n in range(4)
                    ]
                    for k in range(NKD):
                        for n in range(4):
                            nc.tensor.matmul(
                                ps[n][:],
                                w_r[:, k * GD + m * 128: k * GD + m * 128 + 128],
                                ht_r[k][:, n * 512:(n + 1) * 512],
                                start=(k == 0), stop=False,
                            )
                    for n in range(4):
                        nc.tensor.matmul(
                            ps[n][:],
                            brow[:, m * 128:(m + 1) * 128],
                            ones_r[:],
                            start=False, stop=True,
                        )
                        nc.scalar.copy(dest[m][:, n * 512:(n + 1) * 512], ps[n][:])

            # v natural: out[t(128), 260] = sum_k HT_k_t.T @ WvS_k + ones.T @ bvS
            for t in range(NT):
                vp = apsum.tile([128, GV], F32, name="vp", tag="vps", bufs=2)
                for k in range(NKD):
                    nc.tensor.matmul(
                        vp[:],
                        ht_r[k][:, t * 128:(t + 1) * 128],
                        wv_r[:, k * GV:(k + 1) * GV],
                        start=(k == 0), stop=False,
                    )
                nc.tensor.matmul(
                    vp[:], ones_r[:, 0:128], bv_r[:], start=False, stop=True
                )
                nc.scalar.copy(vt[t][:], vp[:])
                nc.vector.tensor_scalar_mul(vt[t][:], vt[t][:], kpm_sb[:, t:t + 1])

        # ================= phase B: attention =================
        with ExitStack() as bctx:
            bpool = bctx.enter_context(tc.tile_pool(name="phB", bufs=1))
            bpsum = bctx.enter_context(tc.tile_pool(name="phB_ps", bufs=1, space="PSUM"))

            def normalize(hp, J, at):
                for hh in range(2):
                    rc = bpool.tile([1, 512], F32, name="rc", tag="rc", bufs=4)
                    nc.vector.reciprocal(rc[:], at[hh][64:65, :])
                    rb = bpool.tile([64, 512], F32, name="rb", tag="rb", bufs=4)
                    nc.gpsimd.partition_broadcast(rb[:], rc[:])
                    nc.vector.tensor_tensor(
                        attT[hp][hh * 64:(hh + 1) * 64, J * 512:(J + 1) * 512],
                        at[hh][0:64, :],
                        rb[:],
                        op=OP.mult,
                    )

            pending_norm = None
            for J in range(NJ):
                for hp in range(2):
                    n_kc = 4 * J + 4
                    at = [
                        bpsum.tile([65, 512], F32, name=f"at{hh}", tag="av", bufs=4)
                        for hh in range(2)
                    ]

                    # task list: diag i=0 first (full width, opens the
                    # accumulation), off-diagonal pairs, then narrow diagonals
                    tasks = [("diag", 4 * J, 0)]
                    for kc in range(0, 4 * J, 2):
                        tasks.append(("wide", kc, kc + 1))
                    for i in range(1, 4):
                        tasks.append(("diag", 4 * J + i, 128 * i))

                    def issue_sc_exp(task):
                        exs = []
                        if task[0] == "wide":
                            _, kc0, kc1 = task
                            for hh in range(2):
                                sc = bpsum.tile([128, 1024], F32, name="sc", tag="sc", bufs=2)
                                for half, kc in ((0, kc0), (1, kc1)):
                                    nc.tensor.matmul(
                                        sc[:, half * 512:(half + 1) * 512],
                                        kT[hp][hh * 64:(hh + 1) * 64, kc * 128:(kc + 1) * 128],
                                        qT[hp][hh * 64:(hh + 1) * 64, J * 512:(J + 1) * 512],
                                        start=True, stop=True,
                                        tile_position=(hh * 64, 0),
                                    )
                                ex = bpool.tile([128, 1024], F32R, name="ex", tag="ex", bufs=4)
                                nc.scalar.activation(ex[:], sc[:], AF.Exp, scale=0.125)
                                exs.append(ex)
                        else:
                            _, kc, off = task
                            w = 512 - off
                            for hh in range(2):
                                sc = bpsum.tile([128, w], F32, name="sc", tag="sc", bufs=2)
                                nc.tensor.matmul(
                                    sc[:],
                                    kT[hp][hh * 64:(hh + 1) * 64, kc * 128:(kc + 1) * 128],
                                    qT[hp][hh * 64:(hh + 1) * 64, J * 512 + off:(J + 1) * 512],
                                    start=True, stop=True,
                                    tile_position=(hh * 64, 0),
                                )
                                ex = bpool.tile([128, w], F32R, name="ex", tag="ex", bufs=4)
                                nc.scalar.activation(ex[:], sc[:], AF.Exp, scale=0.125)
                                # causal band: local cols [0,128)
                                nc.vector.tensor_tensor(
                                    ex[:, 0:128], ex[:, 0:128], tri[:], op=OP.mult
                                )
                                exs.append(ex)
                        return exs

                    def issue_av(task, exs, first, last):
                        if task[0] == "wide":
                            _, kc0, kc1 = task
                            for hh in range(2):
                                h = 2 * hp + hh
                                for half, kc in ((0, kc0), (1, kc1)):
                                    nc.tensor.matmul(
                                        at[hh][:],
                                        vt[kc][:, h * 65:(h + 1) * 65],
                                        exs[hh][:, half * 512:(half + 1) * 512],
                                        start=False,
                                        stop=(last and half == 1),
                                    )
                        else:
                            _, kc, off = task
                            for hh in range(2):
                                h = 2 * hp + hh
                                nc.tensor.matmul(
                                    at[hh][:, off:512],
                                    vt[kc][:, h * 65:(h + 1) * 65],
                                    exs[hh][:],
                                    start=first, stop=last,
                                )

                    # software pipeline: scores/exp one task ahead of AV
                    prev = None
                    for ti, task in enumerate(tasks):
                        exs = issue_sc_exp(task)
                        if prev is not None:
                            issue_av(prev[0], prev[1], first=(prev[2] == 0),
                                     last=False)
                        prev = (task, exs, ti)
                    issue_av(prev[0], prev[1], first=(prev[2] == 0), last=True)

                    # normalize one block late so the slow exact reciprocal
                    # doesn't block this block's DVE mask multiplies
                    if pending_norm is not None:
                        normalize(*pending_norm)
                    pending_norm = (hp, J, at)
            normalize(*pending_norm)

        # ================= phase C: output projection =================
        with ExitStack() as cctx:
            opool = cctx.enter_context(tc.tile_pool(name="phC", bufs=1))
            opsum = cctx.enter_context(tc.tile_pool(name="phC_ps", bufs=1, space="PSUM"))
            for t in range(NT):
                ot = opool.tile([128, D], F32, name="ot", tag="ot", bufs=3)
                for n in range(2):
                    op = opsum.tile([128, 512], F32, name="op", tag="op", bufs=4)
                    for hp in range(2):
                        nc.tensor.matmul(
                            op[:],
                            attT[hp][:, t * 128:(t + 1) * 128],
                            wo_r[hp][:, n * 512:(n + 1) * 512],
                            start=(hp == 0), stop=(hp == 1),
                        )
                    nc.vector.tensor_copy(ot[:, n * 512:(n + 1) * 512], op[:])
                nc.sync.dma_start(O[t * 128:(t + 1) * 128, :], ot[:])

    nc.compile()
    _NC_CACHE["nc"] = nc
    return nc


def _prep_core_inputs(H, key_padding_mask, Wq, bq, Wk, bk, Wv, bv, Wo, bo):
    keep = 1.0 - np.asarray(key_padding_mask, dtype=np.float32)  # [B, T]
    in_maps = []
    for c in range(8):
        b, g = divmod(c, 4)
        sl = slice(g * GD, (g + 1) * GD)
        WvT = Wv[sl].T  # [D, GD]
        WvS = np.zeros((D, GV), dtype=np.float32)
        bvS = np.zeros((1, GV), dtype=np.float32)
        for h in range(HPC):
            WvS[:, h * 65:h * 65 + 64] = WvT[:, h * 64:(h + 1) * 64]
            bvS[0, h * 65:h * 65 + 64] = bv[sl][h * 64:(h + 1) * 64]
            bvS[0, h * 65 + 64] = 1.0
        in_maps.append({
            "HT": np.ascontiguousarray(H[b].T),
            "WqT": np.ascontiguousarray(Wq[sl].T),
            "WkT": np.ascontiguousarray(Wk[sl].T),
            "WvS": WvS,
            "WoS": np.ascontiguousarray(Wo[:, sl].T),
            "bq": np.ascontiguousarray(bq[sl][None, :]),
            "bk": np.ascontiguousarray(bk[sl][None, :]),
            "bvS": bvS,
            "kpm": np.ascontiguousarray(keep[b].reshape(NT, 128).T),
        })
    return in_maps


def kernel(H, key_padding_mask, Wq, bq, Wk, bk, Wv, bv, Wo, bo, _run_kwargs=None):
    H = np.asarray(H, dtype=np.float32)
    Wq = np.asarray(Wq, dtype=np.float32)
    Wk = np.asarray(Wk, dtype=np.float32)
    Wv = np.asarray(Wv, dtype=np.float32)
    Wo = np.asarray(Wo, dtype=np.float32)
    bq = np.asarray(bq, dtype=np.float32)
    bk = np.asarray(bk, dtype=np.float32)
    bv = np.asarray(bv, dtype=np.float32)
    bo = np.asarray(bo, dtype=np.float32)

    nc = build()
    in_maps = _prep_core_inputs(H, key_padding_mask, Wq, bq, Wk, bk, Wv, bv, Wo, bo)
    res = bass_utils.run_bass_kernel_spmd(
        nc, in_maps, core_ids=list(range(8)), **(_run_kwargs or {})
    )
    out = np.zeros((B, T, D), dtype=np.float32)
    for c in range(8):
        out[c // 4] += res.results[c]["O"]
    out += bo
    if _run_kwargs:
        kernel.last_result = res
    return out


# revision 7
# speedup vs baseline: 1.1396x; 1.0602x over previous
"""Multi-head self-attention (B=2, T=2048, D=1024, 16 heads) on 8 TRN2 cores.

Sharding: core c = (b, g) with b = c // 4 (batch), g = c % 4 (head group of 4).
Each core computes q/k/v projections for its 4 heads, causal softmax
attention, and a partial output projection (its 256 columns of the
concat-head dim against Wo). Host sums the 4 partials per batch and adds bo.

All matmuls run in float32r (TF32-like, full PE rate at free dim >= 256).
DRAM inputs are declared float32r so they DMA straight into matmul operands.

Per-core pipeline:
  phase A: qT/kT [256,2048] (transposed projections, head pairs stacked on
           partitions) and v [2048,260] natural (per head 64 value cols + a
           ones col that makes the AV matmul emit softmax denominators).
  phase B: per (head-pair hp, query block J of 512): scoresT chunks
           [tk=128, tq<=512] = kT.T @ qT row-packed via tile_position
           (0,0)/(64,0); ACT exp(0.125 x) PSUM->SBUF (off-diagonal chunks
           paired into [128,1024] PSUM tiles, one ACT instr); diagonal chunks
           column-restricted to the causal region and masked with a single
           [128,128] lower-tri multiply; AV accumulates [v|1].T @ exp into
           attT [65, 512]; row 64 holds denominators; normalize via
           reciprocal_approx_fast + gpsimd partition broadcast + DVE mult.
  phase C: O [2048,1024] = attT.T @ WoS, DVE PSUM->SBUF copies, DMA out.
"""

import numpy as np

import concourse.bass as bass
import concourse.tile as tile
from concourse import bacc, mybir
from concourse import bass_utils
from contextlib import ExitStack

F32 = mybir.dt.float32
F32R = mybir.dt.float32r
BF16 = mybir.dt.bfloat16
ATT = BF16  # dtype for attention-phase matmul operands
AF = mybir.ActivationFunctionType
OP = mybir.AluOpType

B, T, D = 2, 2048, 1024
NH, DH = 16, 64
HPC = 4            # heads per core
GD = HPC * DH      # 256, group dim
GV = HPC * (DH + 1)  # 260, v tile width
NKD = D // 128     # 8 K-chunks for projections
NT = T // 128      # 16 token chunks
NJ = T // 512      # 4 query blocks

_NC_CACHE = {}


def build():
    if "nc" in _NC_CACHE:
        return _NC_CACHE["nc"]
    nc = bacc.Bacc("TRN2", target_bir_lowering=False, debug=False, num_devices=8)

    HT = nc.dram_tensor("HT", [D, T], F32R, kind="ExternalInput").ap()
    WqT = nc.dram_tensor("WqT", [D, GD], F32R, kind="ExternalInput").ap()
    WkT = nc.dram_tensor("WkT", [D, GD], F32R, kind="ExternalInput").ap()
    WvS = nc.dram_tensor("WvS", [D, GV], F32R, kind="ExternalInput").ap()
    WoS = nc.dram_tensor("WoS", [GD, D], F32R, kind="ExternalInput").ap()
    bq = nc.dram_tensor("bq", [1, GD], F32R, kind="ExternalInput").ap()
    bk = nc.dram_tensor("bk", [1, GD], F32R, kind="ExternalInput").ap()
    bvS = nc.dram_tensor("bvS", [1, GV], F32R, kind="ExternalInput").ap()
    kpm = nc.dram_tensor("kpm", [128, NT], F32, kind="ExternalInput").ap()
    O = nc.dram_tensor("O", [T, D], F32, kind="ExternalOutput").ap()

    ENGS = [nc.sync, nc.scalar, nc.gpsimd]

    with tile.TileContext(nc) as tc, ExitStack() as octx:
        cpool = octx.enter_context(tc.tile_pool(name="const", bufs=1))
        keep = octx.enter_context(tc.tile_pool(name="keep", bufs=1))

        # ---- constants ----
        ones_f = cpool.tile([1, 512], F32, name="ones_f", tag="ones_f")
        nc.vector.memset(ones_f[:], 1.0)
        ones_r = cpool.tile([1, 512], F32R, name="ones_r", tag="ones_r")
        nc.vector.tensor_copy(ones_r[:], ones_f[:])

        bq_r = cpool.tile([1, GD], F32R, name="bq_r", tag="bq_r")
        bk_r = cpool.tile([1, GD], F32R, name="bk_r", tag="bk_r")
        bv_r = cpool.tile([1, GV], F32R, name="bv_r", tag="bv_r")
        nc.sync.dma_start(bq_r[:], bq[:])
        nc.sync.dma_start(bk_r[:], bk[:])
        nc.sync.dma_start(bv_r[:], bvS[:])

        kpm_sb = cpool.tile([128, NT], F32, name="kpm_sb", tag="kpm_sb")
        nc.sync.dma_start(kpm_sb[:], kpm[:])

        # lower-tri mask [128,128]: keep where f >= p
        tri = cpool.tile([128, 128], ATT, name="tri", tag="tri")
        nc.gpsimd.memset(tri[:], 1.0)
        nc.gpsimd.affine_select(
            out=tri[:], in_=tri[:], compare_op=OP.is_ge, fill=0.0,
            base=0, pattern=[[1, 128]], channel_multiplier=-1,
        )

        # ---- long-lived activations ----
        qT = [keep.tile([128, T], ATT, name=f"qT{m}", tag=f"qT{m}") for m in range(2)]
        kT = [keep.tile([128, T], ATT, name=f"kT{m}", tag=f"kT{m}") for m in range(2)]
        vt = [keep.tile([128, GV], ATT, name=f"vt{t}", tag=f"vt{t}") for t in range(NT)]
        attT = [keep.tile([128, T], F32R, name=f"attT{m}", tag=f"attT{m}") for m in range(2)]
        wo_r = [keep.tile([128, D], F32R, name=f"wo{i}", tag=f"wo{i}") for i in range(2)]
        for i in range(2):
            nc.scalar.dma_start(wo_r[i][:], WoS[i * 128:(i + 1) * 128, :])

        # ================= phase A: projections =================
        with ExitStack() as actx:
            apool = actx.enter_context(tc.tile_pool(name="phA", bufs=1))
            apsum = actx.enter_context(tc.tile_pool(name="phA_ps", bufs=1, space="PSUM"))

            ht_r = [apool.tile([128, T], F32R, name=f"ht{k}", tag=f"ht{k}") for k in range(NKD)]
            wq_r = apool.tile([128, NKD * GD], F32R, name="wq_r", tag="wq_r")
            wk_r = apool.tile([128, NKD * GD], F32R, name="wk_r", tag="wk_r")
            wv_r = apool.tile([128, NKD * GV], F32R, name="wv_r", tag="wv_r")
            # interleave weight + HT chunk loads across engine DMA queues
            for k in range(NKD):
                e = ENGS[k % len(ENGS)]
                e.dma_start(wq_r[:, k * GD:(k + 1) * GD], WqT[k * 128:(k + 1) * 128, :])
                e.dma_start(wk_r[:, k * GD:(k + 1) * GD], WkT[k * 128:(k + 1) * 128, :])
                e.dma_start(wv_r[:, k * GV:(k + 1) * GV], WvS[k * 128:(k + 1) * 128, :])
                ENGS[(k + 2) % len(ENGS)].dma_start(ht_r[k][:], HT[k * 128:(k + 1) * 128, :])

            # qT / kT: out[dq(128), t(512)] = sum_k WT_k_m.T @ HT_k + bias
            for w_r, dest, brow in ((wq_r, qT, bq_r), (wk_r, kT, bk_r)):
                for m in range(2):
                    ps = [
                        apsum.tile([128, 512], F32, name=f"pp{n}", tag="proj", bufs=6)
                        for n in range(4)
                    ]
                    for k in range(NKD):
                        for n in range(4):
                            nc.tensor.matmul(
                                ps[n][:],
                                w_r[:, k * GD + m * 128: k * GD + m * 128 + 128],
                                ht_r[k][:, n * 512:(n + 1) * 512],
                                start=(k == 0), stop=False,
                            )
                    for n in range(4):
                        nc.tensor.matmul(
                            ps[n][:],
                            brow[:, m * 128:(m + 1) * 128],
                            ones_r[:],
                            start=False, stop=True,
                        )
                        nc.scalar.copy(dest[m][:, n * 512:(n + 1) * 512], ps[n][:])

            # v natural: out[t(128), 260] = sum_k HT_k_t.T @ WvS_k + ones.T @ bvS
            for t in range(NT):
                vp = apsum.tile([128, GV], F32, name="vp", tag="vps", bufs=2)
                for k in range(NKD):
                    nc.tensor.matmul(
                        vp[:],
                        ht_r[k][:, t * 128:(t + 1) * 128],
                        wv_r[:, k * GV:(k + 1) * GV],
                        start=(k == 0), stop=False,
                    )
                nc.tensor.matmul(
                    vp[:], ones_r[:, 0:128], bv_r[:], start=False, stop=True
                )
                nc.scalar.copy(vt[t][:], vp[:])
                nc.vector.tensor_scalar_mul(vt[t][:], vt[t][:], kpm_sb[:, t:t + 1])

        # ================= phase B: attention =================
        with ExitStack() as bctx:
            bpool = bctx.enter_context(tc.tile_pool(name="phB", bufs=1))
            bpsum = bctx.enter_context(tc.tile_pool(name="phB_ps", bufs=1, space="PSUM"))

            def normalize(hp, J, at):
                for hh in range(2):
                    rc = bpool.tile([1, 512], F32, name="rc", tag="rc", bufs=4)
                    nc.vector.reciprocal(rc[:], at[hh][64:65, :])
                    rb = bpool.tile([64, 512], F32, name="rb", tag="rb", bufs=4)
                    nc.gpsimd.partition_broadcast(rb[:], rc[:])
                    nc.vector.tensor_tensor(
                        attT[hp][hh * 64:(hh + 1) * 64, J * 512:(J + 1) * 512],
                        at[hh][0:64, :],
                        rb[:],
                        op=OP.mult,
                    )

            pending_norm = None
            for J in range(NJ):
                for hp in range(2):
                    n_kc = 4 * J + 4
                    at = [
                        bpsum.tile([65, 512], F32, name=f"at{hh}", tag="av", bufs=4)
                        for hh in range(2)
                    ]

                    # task list: diag i=0 first (full width, opens the
                    # accumulation), off-diagonal pairs, then narrow diagonals
                    tasks = [("diag", 4 * J, 0)]
                    for kc in range(0, 4 * J, 2):
                        tasks.append(("wide", kc, kc + 1))
                    for i in range(1, 4):
                        tasks.append(("diag", 4 * J + i, 128 * i))

                    def issue_sc_exp(task):
                        exs = []
                        if task[0] == "wide":
                            _, kc0, kc1 = task
                            scs = [
                                bpsum.tile([128, 1024], F32, name="sc", tag="sc", bufs=2)
                                for _ in range(2)
                            ]
                            for half, kc in ((0, kc0), (1, kc1)):
                                for hh in range(2):
                                    nc.tensor.matmul(
                                        scs[hh][:, half * 512:(half + 1) * 512],
                                        kT[hp][hh * 64:(hh + 1) * 64, kc * 128:(kc + 1) * 128],
                                        qT[hp][hh * 64:(hh + 1) * 64, J * 512:(J + 1) * 512],
                                        start=True, stop=True,
                                        tile_position=(hh * 64, 0),
                                    )
                            for hh in range(2):
                                ex = bpool.tile([128, 1024], ATT, name="ex", tag="ex", bufs=4)
                                nc.scalar.activation(ex[:], scs[hh][:], AF.Exp, scale=0.125)
                                exs.append(ex)
                        else:
                            _, kc, off = task
                            w = 512 - off
                            for hh in range(2):
                                sc = bpsum.tile([128, w], F32, name="sc", tag="sc", bufs=2)
                                nc.tensor.matmul(
                                    sc[:],
                                    kT[hp][hh * 64:(hh + 1) * 64, kc * 128:(kc + 1) * 128],
                                    qT[hp][hh * 64:(hh + 1) * 64, J * 512 + off:(J + 1) * 512],
                                    start=True, stop=True,
                                    tile_position=(hh * 64, 0),
                                )
                                ex = bpool.tile([128, w], ATT, name="ex", tag="ex", bufs=4)
                                nc.scalar.activation(ex[:], sc[:], AF.Exp, scale=0.125)
                                # causal band: local cols [0,128)
                                nc.vector.tensor_tensor(
                                    ex[:, 0:128], ex[:, 0:128], tri[:], op=OP.mult
                                )
                                exs.append(ex)
                        return exs

                    def issue_av(task, exs, first, last):
                        if task[0] == "wide":
                            _, kc0, kc1 = task
                            for hh in range(2):
                                h = 2 * hp + hh
                                for half, kc in ((0, kc0), (1, kc1)):
                                    nc.tensor.matmul(
                                        at[hh][:],
                                        vt[kc][:, h * 65:(h + 1) * 65],
                                        exs[hh][:, half * 512:(half + 1) * 512],
                                        start=False,
                                        stop=(last and half == 1),
                                    )
                        else:
                            _, kc, off = task
                            for hh in range(2):
                                h = 2 * hp + hh
                                nc.tensor.matmul(
                                    at[hh][:, off:512],
                                    vt[kc][:, h * 65:(h + 1) * 65],
                                    exs[hh][:],
                                    start=first, stop=last,
                                )

                    # software pipeline: scores/exp one task ahead of AV
                    prev = None
                    for ti, task in enumerate(tasks):
                        exs = issue_sc_exp(task)
                        if prev is not None:
                            issue_av(prev[0], prev[1], first=(prev[2] == 0),
                                     last=False)
                        prev = (task, exs, ti)
                    issue_av(prev[0], prev[1], first=(prev[2] == 0), last=True)

                    # normalize one block late so the slow exact reciprocal
                    # doesn't block this block's DVE mask multiplies
                    if pending_norm is not None:
                        normalize(*pending_norm)
                    pending_norm = (hp, J, at)
            normalize(*pending_norm)

        # ================= phase C: output projection =================
        with ExitStack() as cctx:
            opool = cctx.enter_context(tc.tile_pool(name="phC", bufs=1))
            opsum = cctx.enter_context(tc.tile_pool(name="phC_ps", bufs=1, space="PSUM"))
            for t in range(NT):
                ot = opool.tile([128, D], F32, name="ot", tag="ot", bufs=3)
                for n in range(2):
                    op = opsum.tile([128, 512], F32, name="op", tag="op", bufs=4)
                    for hp in range(2):
                        nc.tensor.matmul(
                            op[:],
                            attT[hp][:, t * 128:(t + 1) * 128],
                            wo_r[hp][:, n * 512:(n + 1) * 512],
                            start=(hp == 0), stop=(hp == 1),
                        )
                    nc.vector.tensor_copy(ot[:, n * 512:(n + 1) * 512], op[:])
                nc.sync.dma_start(O[t * 128:(t + 1) * 128, :], ot[:])

    nc.compile()
    _NC_CACHE["nc"] = nc
    return nc


def _prep_core_inputs(H, key_padding_mask, Wq, bq, Wk, bk, Wv, bv, Wo, bo):
    keep = 1.0 - np.asarray(key_padding_mask, dtype=np.float32)  # [B, T]
    in_maps = []
    for c in range(8):
        b, g = divmod(c, 4)
        sl = slice(g * GD, (g + 1) * GD)
        WvT = Wv[sl].T  # [D, GD]
        WvS = np.zeros((D, GV), dtype=np.float32)
        bvS = np.zeros((1, GV), dtype=np.float32)
        for h in range(HPC):
            WvS[:, h * 65:h * 65 + 64] = WvT[:, h * 64:(h + 1) * 64]
            bvS[0, h * 65:h * 65 + 64] = bv[sl][h * 64:(h + 1) * 64]
            bvS[0, h * 65 + 64] = 1.0
        in_maps.append({
            "HT": np.ascontiguousarray(H[b].T),
            "WqT": np.ascontiguousarray(Wq[sl].T),
            "WkT": np.ascontiguousarray(Wk[sl].T),
            "WvS": WvS,
            "WoS": np.ascontiguousarray(Wo[:, sl].T),
            "bq": np.ascontiguousarray(bq[sl][None, :]),
            "bk": np.ascontiguousarray(bk[sl][None, :]),
            "bvS": bvS,
            "kpm": np.ascontiguousarray(keep[b].reshape(NT, 128).T),
        })
    return in_maps


def kernel(H, key_padding_mask, Wq, bq, Wk, bk, Wv, bv, Wo, bo, _run_kwargs=None):
    H = np.asarray(H, dtype=np.float32)
    Wq = np.asarray(Wq, dtype=np.float32)
    Wk = np.asarray(Wk, dtype=np.float32)
    Wv = np.asarray(Wv, dtype=np.float32)
    Wo = np.asarray(Wo, dtype=np.float32)
    bq = np.asarray(bq, dtype=np.float32)
    bk = np.asarray(bk, dtype=np.float32)
    bv = np.asarray(bv, dtype=np.float32)
    bo = np.asarray(bo, dtype=np.float32)

    nc = build()
    in_maps = _prep_core_inputs(H, key_padding_mask, Wq, bq, Wk, bk, Wv, bv, Wo, bo)
    res = bass_utils.run_bass_kernel_spmd(
        nc, in_maps, core_ids=list(range(8)), **(_run_kwargs or {})
    )
    out = np.zeros((B, T, D), dtype=np.float32)
    for c in range(8):
        out[c // 4] += res.results[c]["O"]
    out += bo
    if _run_kwargs:
        kernel.last_result = res
    return out


# revision 9
# speedup vs baseline: 1.2666x; 1.1114x over previous
"""Multi-head self-attention (B=2, T=2048, D=1024, 16 heads) on 8 TRN2 cores.

Sharding: core c = (b, g) with b = c // 4 (batch), g = c % 4 (head group of 4).
Each core computes q/k/v projections for its 4 heads, causal softmax
attention, and a partial output projection (its 256 columns of the
concat-head dim against Wo). Host sums the 4 partials per batch and adds bo.

All matmuls run in float32r (TF32-like, full PE rate at free dim >= 256).
DRAM inputs are declared float32r so they DMA straight into matmul operands.

Per-core pipeline:
  phase A: qT/kT [256,2048] (transposed projections, head pairs stacked on
           partitions) and v [2048,260] natural (per head 64 value cols + a
           ones col that makes the AV matmul emit softmax denominators).
  phase B: per (head-pair hp, query block J of 512): scoresT chunks
           [tk=128, tq<=512] = kT.T @ qT row-packed via tile_position
           (0,0)/(64,0); ACT exp(0.125 x) PSUM->SBUF (off-diagonal chunks
           paired into [128,1024] PSUM tiles, one ACT instr); diagonal chunks
           column-restricted to the causal region and masked with a single
           [128,128] lower-tri multiply; AV accumulates [v|1].T @ exp into
           attT [65, 512]; row 64 holds denominators; normalize via
           reciprocal_approx_fast + gpsimd partition broadcast + DVE mult.
  phase C: O [2048,1024] = attT.T @ WoS, DVE PSUM->SBUF copies, DMA out.
"""

import ml_dtypes
import numpy as np

import concourse.bass as bass
import concourse.tile as tile
from concourse import bacc, mybir
from concourse import bass_utils
from contextlib import ExitStack

F32 = mybir.dt.float32
F32R = mybir.dt.float32r
BF16 = mybir.dt.bfloat16
ATT = BF16  # dtype for attention-phase matmul operands
AF = mybir.ActivationFunctionType
OP = mybir.AluOpType

B, T, D = 2, 2048, 1024
NH, DH = 16, 64
HPC = 4            # heads per core
GD = HPC * DH      # 256, group dim
GV = HPC * (DH + 1)  # 260, v tile width
NKD = D // 128     # 8 K-chunks for projections
NT = T // 128      # 16 token chunks
NJ = T // 512      # 4 query blocks

_NC_CACHE = {}


def build():
    if "nc" in _NC_CACHE:
        return _NC_CACHE["nc"]
    nc = bacc.Bacc("TRN2", target_bir_lowering=False, debug=False, num_devices=8)

    HT = nc.dram_tensor("HT", [D, T], BF16, kind="ExternalInput").ap()
    WqT = nc.dram_tensor("WqT", [D, GD], BF16, kind="ExternalInput").ap()
    WkT = nc.dram_tensor("WkT", [D, GD], BF16, kind="ExternalInput").ap()
    WvS = nc.dram_tensor("WvS", [D, GV], BF16, kind="ExternalInput").ap()
    WoS = nc.dram_tensor("WoS", [GD, D], F32R, kind="ExternalInput").ap()
    bq = nc.dram_tensor("bq", [1, GD], BF16, kind="ExternalInput").ap()
    bk = nc.dram_tensor("bk", [1, GD], BF16, kind="ExternalInput").ap()
    bvS = nc.dram_tensor("bvS", [1, GV], BF16, kind="ExternalInput").ap()
    kpm = nc.dram_tensor("kpm", [128, NT], F32, kind="ExternalInput").ap()
    O = nc.dram_tensor("O", [T, D], F32, kind="ExternalOutput").ap()

    ENGS = [nc.sync, nc.scalar, nc.gpsimd]

    with tile.TileContext(nc) as tc, ExitStack() as octx:
        cpool = octx.enter_context(tc.tile_pool(name="const", bufs=1))
        keep = octx.enter_context(tc.tile_pool(name="keep", bufs=1))

        # ---- constants ----
        ones_f = cpool.tile([1, 512], F32, name="ones_f", tag="ones_f")
        nc.vector.memset(ones_f[:], 1.0)
        ones_r = cpool.tile([1, 512], BF16, name="ones_r", tag="ones_r")
        nc.vector.tensor_copy(ones_r[:], ones_f[:])

        bq_r = cpool.tile([1, GD], BF16, name="bq_r", tag="bq_r")
        bk_r = cpool.tile([1, GD], BF16, name="bk_r", tag="bk_r")
        bv_r = cpool.tile([1, GV], BF16, name="bv_r", tag="bv_r")
        nc.sync.dma_start(bq_r[:], bq[:])
        nc.sync.dma_start(bk_r[:], bk[:])
        nc.sync.dma_start(bv_r[:], bvS[:])

        kpm_sb = cpool.tile([128, NT], F32, name="kpm_sb", tag="kpm_sb")
        nc.sync.dma_start(kpm_sb[:], kpm[:])

        # lower-tri mask [128,128]: keep where f >= p
        tri = cpool.tile([128, 128], ATT, name="tri", tag="tri")
        nc.gpsimd.memset(tri[:], 1.0)
        nc.gpsimd.affine_select(
            out=tri[:], in_=tri[:], compare_op=OP.is_ge, fill=0.0,
            base=0, pattern=[[1, 128]], channel_multiplier=-1,
        )

        # ---- long-lived activations ----
        qT = [keep.tile([128, T], ATT, name=f"qT{m}", tag=f"qT{m}") for m in range(2)]
        kT = [keep.tile([128, T], ATT, name=f"kT{m}", tag=f"kT{m}") for m in range(2)]
        vt = [keep.tile([128, GV], ATT, name=f"vt{t}", tag=f"vt{t}") for t in range(NT)]
        attT = [keep.tile([128, T], F32R, name=f"attT{m}", tag=f"attT{m}") for m in range(2)]
        wo_r = [keep.tile([128, D], F32R, name=f"wo{i}", tag=f"wo{i}") for i in range(2)]
        for i in range(2):
            nc.scalar.dma_start(wo_r[i][:], WoS[i * 128:(i + 1) * 128, :])

        # ================= phase A: projections =================
        with ExitStack() as actx:
            apool = actx.enter_context(tc.tile_pool(name="phA", bufs=1))
            apsum = actx.enter_context(tc.tile_pool(name="phA_ps", bufs=1, space="PSUM"))

            ht_r = [apool.tile([128, T], BF16, name=f"ht{k}", tag=f"ht{k}") for k in range(NKD)]
            wq_r = apool.tile([128, NKD * GD], BF16, name="wq_r", tag="wq_r")
            wk_r = apool.tile([128, NKD * GD], BF16, name="wk_r", tag="wk_r")
            wv_r = apool.tile([128, NKD * GV], BF16, name="wv_r", tag="wv_r")
            # interleave weight + HT chunk loads across engine DMA queues
            for k in range(NKD):
                e = ENGS[k % len(ENGS)]
                e.dma_start(wq_r[:, k * GD:(k + 1) * GD], WqT[k * 128:(k + 1) * 128, :])
                e.dma_start(wk_r[:, k * GD:(k + 1) * GD], WkT[k * 128:(k + 1) * 128, :])
                e.dma_start(wv_r[:, k * GV:(k + 1) * GV], WvS[k * 128:(k + 1) * 128, :])
                ENGS[(k + 2) % len(ENGS)].dma_start(ht_r[k][:], HT[k * 128:(k + 1) * 128, :])

            # qT / kT: out[dq(128), t(512)] = sum_k WT_k_m.T @ HT_k + bias
            for w_r, dest, brow in ((wq_r, qT, bq_r), (wk_r, kT, bk_r)):
                for m in range(2):
                    ps = [
                        apsum.tile([128, 512], F32, name=f"pp{n}", tag="proj", bufs=6)
                        for n in range(4)
                    ]
                    for k in range(NKD):
                        for n in range(4):
                            nc.tensor.matmul(
                                ps[n][:],
                                w_r[:, k * GD + m * 128: k * GD + m * 128 + 128],
                                ht_r[k][:, n * 512:(n + 1) * 512],
                                start=(k == 0), stop=False,
                            )
                    for n in range(4):
                        nc.tensor.matmul(
                            ps[n][:],
                            brow[:, m * 128:(m + 1) * 128],
                            ones_r[:],
                            start=False, stop=True,
                        )
                        nc.scalar.copy(dest[m][:, n * 512:(n + 1) * 512], ps[n][:])

            # v natural: out[t(128), 260] = sum_k HT_k_t.T @ WvS_k + ones.T @ bvS
            for t in range(NT):
                vp = apsum.tile([128, GV], F32, name="vp", tag="vps", bufs=2)
                for k in range(NKD):
                    nc.tensor.matmul(
                        vp[:],
                        ht_r[k][:, t * 128:(t + 1) * 128],
                        wv_r[:, k * GV:(k + 1) * GV],
                        start=(k == 0), stop=False,
                    )
                nc.tensor.matmul(
                    vp[:], ones_r[:, 0:128], bv_r[:], start=False, stop=True
                )
                nc.scalar.copy(vt[t][:], vp[:])
                nc.vector.tensor_scalar_mul(vt[t][:], vt[t][:], kpm_sb[:, t:t + 1])

        # ================= phase B: attention =================
        with ExitStack() as bctx:
            bpool = bctx.enter_context(tc.tile_pool(name="phB", bufs=1))
            bpsum = bctx.enter_context(tc.tile_pool(name="phB_ps", bufs=1, space="PSUM"))

            def normalize(hp, J, at):
                for hh in range(2):
                    rc = bpool.tile([1, 512], F32, name="rc", tag="rc", bufs=4)
                    nc.vector.reciprocal(rc[:, 0:256], at[hh][64:65, 0:256])
                    nc.vector.reciprocal(rc[:, 256:512], at[hh][64:65, 256:512])
                    rb = bpool.tile([64, 512], F32, name="rb", tag="rb", bufs=4)
                    nc.gpsimd.partition_broadcast(rb[:], rc[:])
                    nc.vector.tensor_tensor(
                        attT[hp][hh * 64:(hh + 1) * 64, J * 512:(J + 1) * 512],
                        at[hh][0:64, :],
                        rb[:],
                        op=OP.mult,
                    )

            pending_norm = None
            for J in range(NJ):
                for hp in range(2):
                    n_kc = 4 * J + 4
                    at = [
                        bpsum.tile([65, 512], F32, name=f"at{hh}", tag="av", bufs=4)
                        for hh in range(2)
                    ]

                    # task list: diag i=0 first (full width, opens the
                    # accumulation), off-diagonal pairs, then narrow diagonals
                    tasks = [("diag", 4 * J, 0)]
                    for kc in range(0, 4 * J, 2):
                        tasks.append(("wide", kc, kc + 1))
                    for i in range(1, 4):
                        tasks.append(("diag", 4 * J + i, 128 * i))

                    def issue_sc_exp(task):
                        exs = []
                        if task[0] == "wide":
                            _, kc0, kc1 = task
                            scs = [
                                bpsum.tile([128, 1024], F32, name="sc", tag="sc", bufs=2)
                                for _ in range(2)
                            ]
                            for half, kc in ((0, kc0), (1, kc1)):
                                for hh in range(2):
                                    nc.tensor.matmul(
                                        scs[hh][:, half * 512:(half + 1) * 512],
                                        kT[hp][hh * 64:(hh + 1) * 64, kc * 128:(kc + 1) * 128],
                                        qT[hp][hh * 64:(hh + 1) * 64, J * 512:(J + 1) * 512],
                                        start=True, stop=True,
                                        tile_position=(hh * 64, 0),
                                    )
                            for hh in range(2):
                                ex = bpool.tile([128, 1024], ATT, name="ex", tag="ex", bufs=4)
                                nc.scalar.activation(ex[:], scs[hh][:], AF.Exp, scale=0.125)
                                exs.append(ex)
                        else:
                            _, kc, off = task
                            w = 512 - off
                            for hh in range(2):
                                sc = bpsum.tile([128, w], F32, name="sc", tag="sc", bufs=2)
                                nc.tensor.matmul(
                                    sc[:],
                                    kT[hp][hh * 64:(hh + 1) * 64, kc * 128:(kc + 1) * 128],
                                    qT[hp][hh * 64:(hh + 1) * 64, J * 512 + off:(J + 1) * 512],
                                    start=True, stop=True,
                                    tile_position=(hh * 64, 0),
                                )
                                ex = bpool.tile([128, w], ATT, name="ex", tag="ex", bufs=4)
                                nc.scalar.activation(ex[:], sc[:], AF.Exp, scale=0.125)
                                # causal band: local cols [0,128)
                                nc.vector.tensor_tensor(
                                    ex[:, 0:128], ex[:, 0:128], tri[:], op=OP.mult
                                )
                                exs.append(ex)
                        return exs

                    def issue_av(task, exs, first, last):
                        if task[0] == "wide":
                            _, kc0, kc1 = task
                            for hh in range(2):
                                h = 2 * hp + hh
                                for half, kc in ((0, kc0), (1, kc1)):
                                    nc.tensor.matmul(
                                        at[hh][:],
                                        vt[kc][:, h * 65:(h + 1) * 65],
                                        exs[hh][:, half * 512:(half + 1) * 512],
                                        start=False,
                                        stop=(last and half == 1),
                                    )
                        else:
                            _, kc, off = task
                            for hh in range(2):
                                h = 2 * hp + hh
                                nc.tensor.matmul(
                                    at[hh][:, off:512],
                                    vt[kc][:, h * 65:(h + 1) * 65],
                                    exs[hh][:],
                                    start=first, stop=last,
                                )

                    # software pipeline: scores/exp one task ahead of AV;
                    # the previous block's normalize chain is issued after the
                    # first task so its slow reciprocal overlaps the mask-free
                    # off-diagonal stretch instead of blocking DVE masks
                    prev = None
                    for ti, task in enumerate(tasks):
                        exs = issue_sc_exp(task)
                        if ti == 1 and pending_norm is not None:
                            normalize(*pending_norm)
                            pending_norm = None
                        if prev is not None:
                            issue_av(prev[0], prev[1], first=(prev[2] == 0),
                                     last=False)
                        prev = (task, exs, ti)
                    issue_av(prev[0], prev[1], first=(prev[2] == 0), last=True)

                    if pending_norm is not None:
                        normalize(*pending_norm)
                    pending_norm = (hp, J, at)
            normalize(*pending_norm)

        # ================= phase C: output projection =================
        with ExitStack() as cctx:
            opool = cctx.enter_context(tc.tile_pool(name="phC", bufs=1))
            opsum = cctx.enter_context(tc.tile_pool(name="phC_ps", bufs=1, space="PSUM"))
            for t in range(NT):
                ot = opool.tile([128, D], F32, name="ot", tag="ot", bufs=3)
                for n in range(2):
                    op = opsum.tile([128, 512], F32, name="op", tag="op", bufs=4)
                    for hp in range(2):
                        nc.tensor.matmul(
                            op[:],
                            attT[hp][:, t * 128:(t + 1) * 128],
                            wo_r[hp][:, n * 512:(n + 1) * 512],
                            start=(hp == 0), stop=(hp == 1),
                        )
                    nc.vector.tensor_copy(ot[:, n * 512:(n + 1) * 512], op[:])
                nc.sync.dma_start(O[t * 128:(t + 1) * 128, :], ot[:])

    nc.compile()
    _NC_CACHE["nc"] = nc
    return nc


def _prep_core_inputs(H, key_padding_mask, Wq, bq, Wk, bk, Wv, bv, Wo, bo):
    keep = 1.0 - np.asarray(key_padding_mask, dtype=np.float32)  # [B, T]
    in_maps = []
    for c in range(8):
        b, g = divmod(c, 4)
        sl = slice(g * GD, (g + 1) * GD)
        WvT = Wv[sl].T  # [D, GD]
        WvS = np.zeros((D, GV), dtype=np.float32)
        bvS = np.zeros((1, GV), dtype=np.float32)
        for h in range(HPC):
            WvS[:, h * 65:h * 65 + 64] = WvT[:, h * 64:(h + 1) * 64]
            bvS[0, h * 65:h * 65 + 64] = bv[sl][h * 64:(h + 1) * 64]
            bvS[0, h * 65 + 64] = 1.0
        bf = ml_dtypes.bfloat16
        in_maps.append({
            "HT": np.ascontiguousarray(H[b].T).astype(bf),
            "WqT": np.ascontiguousarray(Wq[sl].T).astype(bf),
            "WkT": np.ascontiguousarray(Wk[sl].T).astype(bf),
            "WvS": WvS.astype(bf),
            "WoS": np.ascontiguousarray(Wo[:, sl].T),
            "bq": np.ascontiguousarray(bq[sl][None, :]).astype(bf),
            "bk": np.ascontiguousarray(bk[sl][None, :]).astype(bf),
            "bvS": bvS.astype(bf),
            "kpm": np.ascontiguousarray(keep[b].reshape(NT, 128).T),
        })
    return in_maps


def kernel(H, key_padding_mask, Wq, bq, Wk, bk, Wv, bv, Wo, bo, _run_kwargs=None):
    H = np.asarray(H, dtype=np.float32)
    Wq = np.asarray(Wq, dtype=np.float32)
    Wk = np.asarray(Wk, dtype=np.float32)
    Wv = np.asarray(Wv, dtype=np.float32)
    Wo = np.asarray(Wo, dtype=np.float32)
    bq = np.asarray(bq, dtype=np.float32)
    bk = np.asarray(bk, dtype=np.float32)
    bv = np.asarray(bv, dtype=np.float32)
    bo = np.asarray(bo, dtype=np.float32)

    nc = build()
    in_maps = _prep_core_inputs(H, key_padding_mask, Wq, bq, Wk, bk, Wv, bv, Wo, bo)
    res = bass_utils.run_bass_kernel_spmd(
        nc, in_maps, core_ids=list(range(8)), **(_run_kwargs or {})
    )
    out = np.zeros((B, T, D), dtype=np.float32)
    for c in range(8):
        out[c // 4] += res.results[c]["O"]
    out += bo
    if _run_kwargs:
        kernel.last_result = res
    return out
